# revision 13
# baseline (speedup 1.0000x reference)
"""Bias-augmented attention (AlphaFold-style) on 8 Trainium2 NeuronCores.

Problem: B=1, Q=K=2048, C_IN=256, H=8, CH=32
    q = (q_x @ w_q) / sqrt(CH); k = kv_x @ w_k; v = kv_x @ w_v   (per head)
    a = softmax(q k^T + pair_bias + mask_bias)
    o = (a v) * sigmoid(q_x @ w_g + b_g)
    out = o @ w_o + b_o

Sharding: data-parallel over query rows. Core i handles q rows
[256*i, 256*(i+1)), all 8 heads. Per-core HBM traffic ~19.3MB (16.8MB of
which is its pair_bias slice), the minimum for this sharding.

Per-core kernel layout choices:
  * Scores are computed transposed (S^T[k, q], k on PSUM partitions) so the
    A@V contraction (over k) needs no on-chip transposes. pair_bias is
    pre-transposed per-shard on the host (layout prep during sharding).
  * softmax denominator: V is augmented with a ones-column (M=33), so one
    accumulating matmul chain produces both A-numerator@V and the denominator.
  * mask_bias folds in as exp(mask)[k] scaling of V-hat rows (k is the
    partition dim of V-hat, so it is a free per-partition scalar multiply
    fused into the PSUM evacuation copy).
  * 1/sqrt(CH) is folded into w_q on the host.
  * The 1/denominator[q] factor commutes past gating and the d-contraction;
    it is broadcast across partitions with a tiny PE outer-product and
    applied right before the output projection.
  * Matmuls run in float32r (full PE rate at moving-dim>=256); exp output is
    bf16 which the A@V matmul consumes directly.
"""

import math
import sys

for _p in ("/opt/trn_rl_repo",):
    if _p not in sys.path:
        sys.path.insert(0, _p)

import numpy as np

import concourse.bass as bass
import concourse.mybir as mybir
import concourse.tile as tile
from concourse import bacc
from concourse.bass_utils import run_bass_kernel_spmd

F32 = mybir.dt.float32
F32R = mybir.dt.float32r
BF16 = mybir.dt.bfloat16

B, Q, K, C, H, CH = 1, 2048, 2048, 256, 8, 32
NCORES = 8
QS = Q // NCORES  # 256 query rows per core
KC = K // 128  # 16 key chunks of 128
GK = 4  # k-chunks per streaming group
NG = KC // GK  # 4 groups per head


def r32(ap):
    return ap.bitcast(F32R)


def build_nc():
    nc = bacc.Bacc("TRN2", target_bir_lowering=False, debug=False)

    # ---- DRAM I/O (per-core shard shapes) ----
    pairT = nc.dram_tensor("pairT", [H, K, QS], F32, kind="ExternalInput").ap()
    qxT = nc.dram_tensor("qxT", [C, QS], F32, kind="ExternalInput").ap()
    kvxT = nc.dram_tensor("kvxT", [C, K], F32, kind="ExternalInput").ap()
    wq = nc.dram_tensor("wq", [C, C], F32, kind="ExternalInput").ap()  # pre-scaled
    wk = nc.dram_tensor("wk", [C, C], F32, kind="ExternalInput").ap()
    wv = nc.dram_tensor("wv", [C, C], F32, kind="ExternalInput").ap()
    wg = nc.dram_tensor("wg", [C, C], F32, kind="ExternalInput").ap()
    wo = nc.dram_tensor("wo", [C, C], F32, kind="ExternalInput").ap()
    bgt = nc.dram_tensor("bgt", [CH, H], F32, kind="ExternalInput").ap()
    maskT = nc.dram_tensor("maskT", [128, KC], F32, kind="ExternalInput").ap()
    ones_d = nc.dram_tensor("ones", [1, CH], F32, kind="ExternalInput").ap()
    y = nc.dram_tensor("y", [QS, C], F32, kind="ExternalOutput").ap()

    with tile.TileContext(nc) as tc:
        with (
            tc.tile_pool(name="const", bufs=1) as const_pool,
            tc.tile_pool(name="proj", bufs=2) as proj_pool,
            tc.tile_pool(name="stream", bufs=6) as stream_pool,
            tc.tile_pool(name="exps", bufs=4) as exp_pool,
            tc.tile_pool(name="head", bufs=2) as head_pool,
            tc.tile_pool(name="mm", bufs=3, space="PSUM") as mmsum,
            tc.tile_pool(name="otsum", bufs=1, space="PSUM") as otsum_pool,
            tc.tile_pool(name="ysum", bufs=1, space="PSUM") as ysum_pool,
        ):
            # ---- constants / static operands in SBUF ----
            def load_f32r(name, ap, shape):
                t = const_pool.tile(shape, F32R, tag=name)
                nc.sync.dma_start(out=t, in_=r32(ap))
                return t

            # weights, split along contraction dim c into 2 strips of 128
            wq_s = [load_f32r(f"wq{s}", wq[128 * s : 128 * (s + 1), :], [128, C]) for s in range(2)]
            wk_s = [load_f32r(f"wk{s}", wk[128 * s : 128 * (s + 1), :], [128, C]) for s in range(2)]
            wv_s = [load_f32r(f"wv{s}", wv[128 * s : 128 * (s + 1), :], [128, C]) for s in range(2)]
            wg_s = [load_f32r(f"wg{s}", wg[128 * s : 128 * (s + 1), :], [128, C]) for s in range(2)]
            qxT_s = [load_f32r(f"qxT{s}", qxT[128 * s : 128 * (s + 1), :], [128, QS]) for s in range(2)]
            kvxT_s = [load_f32r(f"kvxT{s}", kvxT[128 * s : 128 * (s + 1), :], [128, K]) for s in range(2)]
            # per-head w_o slice [32, 256] (d on partitions)
            wo_h = [load_f32r(f"wo{h}", wo[CH * h : CH * (h + 1), :], [CH, C]) for h in range(H)]

            bgt_sb = const_pool.tile([CH, H], F32, tag="bgt")
            nc.sync.dma_start(out=bgt_sb, in_=bgt)
            maskT_sb = const_pool.tile([128, KC], F32, tag="maskT")
            nc.sync.dma_start(out=maskT_sb, in_=maskT)
            # per-head gate gT[h][d, q] = sigmoid((q_x @ w_g)^T + b_g)
            gT = []
            for h in range(H):
                g_t = const_pool.tile([CH, QS], F32, tag=f"gT{h}")
                ps = mmsum.tile([128, 1024], F32, tag="sp", name="ps")[0:CH, 0:QS]
                for s in range(2):
                    nc.tensor.matmul(
                        ps,
                        wg_s[s][:, CH * h : CH * (h + 1)],
                        qxT_s[s],
                        start=(s == 0),
                        stop=(s == 1),
                    )
                nc.scalar.activation(
                    out=g_t,
                    in_=ps,
                    func=mybir.ActivationFunctionType.Sigmoid,
                    bias=bgt_sb[:, h : h + 1],
                )
                gT.append(g_t)

            em = const_pool.tile([128, KC], F32, tag="em")  # exp(mask_bias) per k
            nc.scalar.activation(out=em, in_=maskT_sb, func=mybir.ActivationFunctionType.Exp)

            ones_t = const_pool.tile([CH + 1, CH], F32R, tag="ones")
            nc.sync.dma_start(out=ones_t[CH : CH + 1, :], in_=r32(ones_d))

            # ---- projections ----
            # kT[t][32*(h%4)+d, kpos] = K[kpos, 32*(4t+h%4)+d], t = h//4
            kT = []
            for t in range(2):
                kT_t = const_pool.tile([128, K], BF16, tag=f"kT{t}")
                for n in range(K // 512):
                    ps = mmsum.tile([128, 1024], F32, tag="sp", name="ps")[:, 0:512]
                    for s in range(2):
                        nc.tensor.matmul(
                            ps,
                            wk_s[s][:, 128 * t : 128 * (t + 1)],
                            kvxT_s[s][:, 512 * n : 512 * (n + 1)],
                            start=(s == 0),
                            stop=(s == 1),
                        )
                    nc.vector.tensor_copy(kT_t[:, 512 * n : 512 * (n + 1)], ps)
                kT.append(kT_t)

            # qT[t][32*(h%4)+d, q] (w_q pre-scaled by 1/sqrt(CH))
            qT = []
            for t in range(2):
                qT_t = const_pool.tile([128, QS], BF16, tag=f"qT{t}")
                ps = mmsum.tile([128, 1024], F32, tag="sp", name="ps")[:, 0:QS]
                for s in range(2):
                    nc.tensor.matmul(
                        ps,
                        wq_s[s][:, 128 * t : 128 * (t + 1)],
                        qxT_s[s],
                        start=(s == 0),
                        stop=(s == 1),
                    )
                nc.vector.tensor_copy(qT_t, ps)
                qT.append(qT_t)

            # vhat[c][p, h, 0:32] = V[128c+p, 32h+d] * exp(mask)[128c+p]
            # vhat[c][p, h, 32]   = exp(mask)[128c+p]
            vhat = []
            for c in range(KC):
                vh = const_pool.tile([128, H, CH + 1], BF16, tag=f"vhat{c}")
                ps = mmsum.tile([128, 1024], F32, tag="sp", name="ps")[:, 0:C]
                for s in range(2):
                    nc.tensor.matmul(
                        ps,
                        kvxT_s[s][:, 128 * c : 128 * (c + 1)],
                        wv_s[s],
                        start=(s == 0),
                        stop=(s == 1),
                    )
                emc = em[:, c : c + 1]
                nc.vector.tensor_scalar_mul(
                    vh[:, :, 0:CH], ps.rearrange("p (h d) -> p h d", h=H), emc
                )
                nc.vector.tensor_copy(vh[:, :, CH : CH + 1], emc.broadcast_to((128, H, 1)))
                vhat.append(vh)

            # ---- output accumulator [QS, C] as one PSUM bank ----
            y_ps = ysum_pool.tile([128, 512], F32, tag="y")

            # ---- streaming attention, head-major ----
            for h in range(H):
                t, hh = h // 4, h % 4
                ot = otsum_pool.tile([CH + 1, QS], F32, tag="ot")
                for g in range(NG):
                    # pair_bias^T tile for 4 k-chunks: [128, 4, QS]
                    pt = stream_pool.tile([128, GK, QS], F32, tag="pt")
                    nc.sync.dma_start(
                        out=pt,
                        in_=pairT[h, 512 * g : 512 * (g + 1), :].rearrange(
                            "(j p) q -> p j q", p=128
                        ),
                    )
                    sp = mmsum.tile([128, GK * QS], F32, tag="sp")
                    for j in range(GK):
                        c = GK * g + j
                        nc.tensor.matmul(
                            sp[:, QS * j : QS * (j + 1)],
                            kT[t][32 * hh : 32 * hh + 32, 128 * c : 128 * (c + 1)],
                            qT[t][32 * hh : 32 * hh + 32, :],
                            start=True,
                            stop=True,
                            tile_position=(32 * hh, 0),
                        )
                    # S^T += pair^T ; E = exp(.) in bf16
                    nc.vector.tensor_add(sp, sp, pt.rearrange("p j q -> p (j q)"))
                    e_t = exp_pool.tile([128, GK * QS], BF16, tag="E")
                    nc.scalar.activation(out=e_t, in_=sp, func=mybir.ActivationFunctionType.Exp)
                    for j in range(GK):
                        c = GK * g + j
                        nc.tensor.matmul(
                            ot,
                            vhat[c][:, h, :],
                            e_t[:, QS * j : QS * (j + 1)],
                            start=(c == 0),
                            stop=(c == KC - 1),
                            skip_group_check=True,
                        )
                # ot rows 0..31 = unnormalized (A@V)^T, row 32 = denominator
                # NB: reciprocal_approx_* are custom DVE opcodes that crash
                # this runtime (NRT_EXEC_UNIT_UNRECOVERABLE); use the exact one.
                r1f = head_pool.tile([CH + 1, QS], F32, tag="r1f")
                nc.vector.reciprocal(out=r1f[CH : CH + 1, :], in_=ot[CH : CH + 1, :])
                r1 = head_pool.tile([CH + 1, QS], F32R, tag="r1")
                nc.vector.tensor_copy(r1[CH : CH + 1, :], r32(r1f)[CH : CH + 1, :])
                rb = mmsum.tile([128, 1024], F32, tag="sp", name="ps")[0:CH, 0:QS]
                nc.tensor.matmul(
                    rb,
                    ones_t[CH : CH + 1, :],
                    r1[CH : CH + 1, :],
                    start=True,
                    stop=True,
                )
                gom = head_pool.tile([CH, QS], F32R, tag="gom")
                with nc.allow_low_precision(reason="f32r is fp32-width for PE lhsT"):
                    nc.vector.tensor_mul(gom, ot[0:CH, :], gT[h])
                    nc.vector.tensor_mul(gom, gom, r32(rb))
                for qc in range(QS // 128):
                    nc.tensor.matmul(
                        y_ps[:, 256 * qc : 256 * (qc + 1)],
                        gom[:, 128 * qc : 128 * (qc + 1)],
                        wo_h[h],
                        # start=True clears has_written for the WHOLE bank, so
                        # only the very first matmul into this bank may set it;
                        # the qc=1 region then overwrites via per-element bits.
                        start=(h == 0 and qc == 0),
                        stop=(h == H - 1),
                        skip_group_check=True,
                    )

            # ---- evacuate y ----
            for qc in range(QS // 128):
                ys = head_pool.tile([128, C], F32, tag="ys")
                nc.vector.tensor_copy(ys, y_ps[:, 256 * qc : 256 * (qc + 1)])
                nc.sync.dma_start(out=y[128 * qc : 128 * (qc + 1), :], in_=ys)

    nc.compile()
    return nc


_NC_CACHE = None


def get_nc():
    global _NC_CACHE
    if _NC_CACHE is None:
        _NC_CACHE = build_nc()
    return _NC_CACHE


def make_in_maps(q_x, kv_x, pair_bias, mask_bias, w_q, w_k, w_v, w_g, b_g, w_o):
    f = np.float32
    q_x = np.asarray(q_x, f)
    kv_x = np.asarray(kv_x, f)
    pair_bias = np.asarray(pair_bias, f)
    mask_bias = np.asarray(mask_bias, f)
    shared = {
        "kvxT": np.ascontiguousarray(kv_x[0].T),
        "wq": np.ascontiguousarray(np.asarray(w_q, f) / math.sqrt(CH)),
        "wk": np.ascontiguousarray(np.asarray(w_k, f)),
        "wv": np.ascontiguousarray(np.asarray(w_v, f)),
        "wg": np.ascontiguousarray(np.asarray(w_g, f)),
        "wo": np.ascontiguousarray(np.asarray(w_o, f)),
        "bgt": np.ascontiguousarray(np.asarray(b_g, f).reshape(H, CH).T),
        "maskT": np.ascontiguousarray(mask_bias.reshape(KC, 128).T),
        "ones": np.ones((1, CH), f),
    }
    in_maps = []
    for i in range(NCORES):
        sl = slice(QS * i, QS * (i + 1))
        in_maps.append(
            dict(
                shared,
                pairT=np.ascontiguousarray(pair_bias[0, :, sl, :].transpose(0, 2, 1)),
                qxT=np.ascontiguousarray(q_x[0, sl, :].T),
            )
        )
    return in_maps


def kernel(
    q_x, kv_x, pair_bias, mask_bias, w_q, w_k, w_v, w_g, b_g, w_o, b_o, **run_kwargs
):
    nc = get_nc()
    in_maps = make_in_maps(
        q_x, kv_x, pair_bias, mask_bias, w_q, w_k, w_v, w_g, b_g, w_o
    )
    res = run_bass_kernel_spmd(nc, in_maps, core_ids=list(range(NCORES)), **run_kwargs)
    out = np.concatenate([res.results[i]["y"] for i in range(NCORES)], axis=0)
    out = out + np.asarray(b_o, np.float32)[None, :]
    kernel.last_result = res
    return out[None].astype(np.float32)


# revision 16
# speedup vs baseline: 1.0496x; 1.0496x over previous
"""Bias-augmented attention (AlphaFold-style) on 8 Trainium2 NeuronCores.

Problem: B=1, Q=K=2048, C_IN=256, H=8, CH=32
    q = (q_x @ w_q) / sqrt(CH); k = kv_x @ w_k; v = kv_x @ w_v   (per head)
    a = softmax(q k^T + pair_bias + mask_bias)
    o = (a v) * sigmoid(q_x @ w_g + b_g)
    out = o @ w_o + b_o

Sharding: data-parallel over query rows. Core i handles q rows
[256*i, 256*(i+1)), all 8 heads. Per-core HBM traffic ~19.3MB (16.8MB of
which is its pair_bias slice), the minimum for this sharding.

Per-core kernel layout choices:
  * Scores are computed transposed (S^T[k, q], k on PSUM partitions) so the
    A@V contraction (over k) needs no on-chip transposes. pair_bias is
    pre-transposed per-shard on the host (layout prep during sharding).
  * softmax denominator: V is augmented with a ones-column (M=33), so one
    accumulating matmul chain produces both A-numerator@V and the denominator.
  * mask_bias folds in as exp(mask)[k] scaling of V-hat rows (k is the
    partition dim of V-hat, so it is a free per-partition scalar multiply
    fused into the PSUM evacuation copy).
  * 1/sqrt(CH) is folded into w_q on the host.
  * The 1/denominator[q] factor commutes past gating and the d-contraction;
    it is broadcast across partitions with a tiny PE outer-product and
    applied right before the output projection.
  * Matmuls run in float32r (full PE rate at moving-dim>=256); exp output is
    bf16 which the A@V matmul consumes directly.
"""

import math
import sys

for _p in ("/opt/trn_rl_repo",):
    if _p not in sys.path:
        sys.path.insert(0, _p)

import numpy as np

import concourse.bass as bass
import concourse.mybir as mybir
import concourse.tile as tile
from concourse import bacc
from concourse.bass_utils import run_bass_kernel_spmd

F32 = mybir.dt.float32
F32R = mybir.dt.float32r
BF16 = mybir.dt.bfloat16

B, Q, K, C, H, CH = 1, 2048, 2048, 256, 8, 32
NCORES = 8
QS = Q // NCORES  # 256 query rows per core
KC = K // 128  # 16 key chunks of 128
GK = 4  # k-chunks per streaming group
NG = KC // GK  # 4 groups per head


def r32(ap):
    return ap.bitcast(F32R)


def build_nc():
    nc = bacc.Bacc("TRN2", target_bir_lowering=False, debug=False)

    # ---- DRAM I/O (per-core shard shapes) ----
    pairT = nc.dram_tensor("pairT", [H, K, QS], F32, kind="ExternalInput").ap()
    qxT = nc.dram_tensor("qxT", [C, QS], F32, kind="ExternalInput").ap()
    kvxT = nc.dram_tensor("kvxT", [C, K], F32, kind="ExternalInput").ap()
    wq = nc.dram_tensor("wq", [C, C], F32, kind="ExternalInput").ap()  # pre-scaled
    wk = nc.dram_tensor("wk", [C, C], F32, kind="ExternalInput").ap()
    wv = nc.dram_tensor("wv", [C, C], F32, kind="ExternalInput").ap()
    wg = nc.dram_tensor("wg", [C, C], F32, kind="ExternalInput").ap()
    wo = nc.dram_tensor("wo", [C, C], F32, kind="ExternalInput").ap()
    bgt = nc.dram_tensor("bgt", [CH, H], F32, kind="ExternalInput").ap()
    maskT = nc.dram_tensor("maskT", [128, KC], F32, kind="ExternalInput").ap()
    ones_d = nc.dram_tensor("ones", [1, CH], F32, kind="ExternalInput").ap()
    ident_d = nc.dram_tensor("ident", [128, 128], F32, kind="ExternalInput").ap()
    y = nc.dram_tensor("y", [QS, C], F32, kind="ExternalOutput").ap()

    with tile.TileContext(nc) as tc:
        with (
            tc.tile_pool(name="const", bufs=1) as const_pool,
            tc.tile_pool(name="proj", bufs=2) as proj_pool,
            tc.tile_pool(name="stream", bufs=6) as stream_pool,
            tc.tile_pool(name="exps", bufs=4) as exp_pool,
            tc.tile_pool(name="head", bufs=2) as head_pool,
            tc.tile_pool(name="mm", bufs=3, space="PSUM") as mmsum,
            tc.tile_pool(name="otsum", bufs=1, space="PSUM") as otsum_pool,
            tc.tile_pool(name="ysum", bufs=1, space="PSUM") as ysum_pool,
        ):
            # ---- constants / static operands in SBUF ----
            def load_f32r(name, ap, shape):
                t = const_pool.tile(shape, F32R, tag=name)
                nc.sync.dma_start(out=t, in_=r32(ap))
                return t

            # weights, split along contraction dim c into 2 strips of 128
            wq_s = [load_f32r(f"wq{s}", wq[128 * s : 128 * (s + 1), :], [128, C]) for s in range(2)]
            wk_s = [load_f32r(f"wk{s}", wk[128 * s : 128 * (s + 1), :], [128, C]) for s in range(2)]
            wv_s = [load_f32r(f"wv{s}", wv[128 * s : 128 * (s + 1), :], [128, C]) for s in range(2)]
            wg_s = [load_f32r(f"wg{s}", wg[128 * s : 128 * (s + 1), :], [128, C]) for s in range(2)]
            qxT_s = [load_f32r(f"qxT{s}", qxT[128 * s : 128 * (s + 1), :], [128, QS]) for s in range(2)]
            kvxT_s = [load_f32r(f"kvxT{s}", kvxT[128 * s : 128 * (s + 1), :], [128, K]) for s in range(2)]
            # per-head w_o slice [32, 256] (d on partitions)
            wo_h = [load_f32r(f"wo{h}", wo[CH * h : CH * (h + 1), :], [CH, C]) for h in range(H)]

            bgt_sb = const_pool.tile([CH, H], F32, tag="bgt")
            nc.sync.dma_start(out=bgt_sb, in_=bgt)
            maskT_sb = const_pool.tile([128, KC], F32, tag="maskT")
            nc.sync.dma_start(out=maskT_sb, in_=maskT)
            # per-head gate gT[h][d, q] = sigmoid((q_x @ w_g)^T + b_g)
            gT = []
            for h in range(H):
                g_t = const_pool.tile([CH, QS], F32, tag=f"gT{h}")
                ps = mmsum.tile([128, 1024], F32, tag="sp", name="ps")[0:CH, 0:QS]
                for s in range(2):
                    nc.tensor.matmul(
                        ps,
                        wg_s[s][:, CH * h : CH * (h + 1)],
                        qxT_s[s],
                        start=(s == 0),
                        stop=(s == 1),
                    )
                nc.scalar.activation(
                    out=g_t,
                    in_=ps,
                    func=mybir.ActivationFunctionType.Sigmoid,
                    bias=bgt_sb[:, h : h + 1],
                )
                gT.append(g_t)

            em = const_pool.tile([128, KC], F32, tag="em")  # exp(mask_bias) per k
            nc.scalar.activation(out=em, in_=maskT_sb, func=mybir.ActivationFunctionType.Exp)

            ones_t = const_pool.tile([CH + 1, CH], F32R, tag="ones")
            nc.sync.dma_start(out=ones_t[CH : CH + 1, :], in_=r32(ones_d))
            ident_t = const_pool.tile([128, 128], F32R, tag="ident")
            nc.sync.dma_start(out=ident_t, in_=r32(ident_d))

            # ---- projections ----
            # kT[t][32*(h%4)+d, kpos] = K[kpos, 32*(4t+h%4)+d], t = h//4
            kT = []
            for t in range(2):
                kT_t = const_pool.tile([128, K], BF16, tag=f"kT{t}")
                for n in range(K // 512):
                    ps = mmsum.tile([128, 1024], F32, tag="sp", name="ps")[:, 0:512]
                    for s in range(2):
                        nc.tensor.matmul(
                            ps,
                            wk_s[s][:, 128 * t : 128 * (t + 1)],
                            kvxT_s[s][:, 512 * n : 512 * (n + 1)],
                            start=(s == 0),
                            stop=(s == 1),
                        )
                    nc.vector.tensor_copy(kT_t[:, 512 * n : 512 * (n + 1)], ps)
                kT.append(kT_t)

            # qT[t][32*(h%4)+d, q] (w_q pre-scaled by 1/sqrt(CH))
            qT = []
            for t in range(2):
                qT_t = const_pool.tile([128, QS], BF16, tag=f"qT{t}")
                ps = mmsum.tile([128, 1024], F32, tag="sp", name="ps")[:, 0:QS]
                for s in range(2):
                    nc.tensor.matmul(
                        ps,
                        wq_s[s][:, 128 * t : 128 * (t + 1)],
                        qxT_s[s],
                        start=(s == 0),
                        stop=(s == 1),
                    )
                nc.vector.tensor_copy(qT_t, ps)
                qT.append(qT_t)

            # vhat[c][p, h, 0:32] = V[128c+p, 32h+d] * exp(mask)[128c+p]
            # vhat[c][p, h, 32]   = exp(mask)[128c+p]
            vhat = []
            for c in range(KC):
                vh = const_pool.tile([128, H, CH + 1], BF16, tag=f"vhat{c}")
                ps = mmsum.tile([128, 1024], F32, tag="sp", name="ps")[:, 0:C]
                for s in range(2):
                    nc.tensor.matmul(
                        ps,
                        kvxT_s[s][:, 128 * c : 128 * (c + 1)],
                        wv_s[s],
                        start=(s == 0),
                        stop=(s == 1),
                    )
                emc = em[:, c : c + 1]
                nc.vector.tensor_scalar_mul(
                    vh[:, :, 0:CH], ps.rearrange("p (h d) -> p h d", h=H), emc
                )
                nc.vector.tensor_copy(vh[:, :, CH : CH + 1], emc.broadcast_to((128, H, 1)))
                vhat.append(vh)

            # ---- output accumulator [QS, C] as one PSUM bank ----
            y_ps = ysum_pool.tile([128, 512], F32, tag="y")

            # ---- streaming attention, software-pipelined ----
            # Emission order is the scheduler's priority hint. The key idea:
            # each step emits the NEXT group's QK matmuls before the PREVIOUS
            # group's A@V matmuls, so the PE never sits behind a wait for this
            # group's DVE-add + ACT-exp. Per-head tail work (reciprocal,
            # broadcast, gate, output projection) is spread over the following
            # steps the same way. Half the groups fold pair_bias into PSUM via
            # identity-matmul accumulation (PE), the other half via DVE adds,
            # balancing the two engines.
            steps = [(h, g) for h in range(H) for g in range(NG)]
            pending_av = None
            tail_queue = []
            ot_by_head = {}

            def emit_qk(i):
                h, g = steps[i]
                t, hh = h // 4, h % 4
                pt = stream_pool.tile([128, GK, QS], F32R, tag="pt", name="pt")
                nc.sync.dma_start(
                    out=pt,
                    in_=r32(pairT)[h, 512 * g : 512 * (g + 1), :].rearrange(
                        "(j p) q -> p j q", p=128
                    ),
                )
                sp = mmsum.tile([128, GK * QS], F32, tag="sp", name="sp")
                for j in range(GK):
                    c = GK * g + j
                    nc.tensor.matmul(
                        sp[:, QS * j : QS * (j + 1)],
                        kT[t][32 * hh : 32 * hh + 32, 128 * c : 128 * (c + 1)],
                        qT[t][32 * hh : 32 * hh + 32, :],
                        # one bank-bit clear per PSUM bank (j=0,1 share a bank)
                        start=(j % 2 == 0),
                        stop=True,
                        tile_position=(32 * hh, 0),
                        skip_group_check=True,
                    )
                pt_flat = pt.rearrange("p j q -> p (j q)")
                import os as _os
                if i % 2 == 0 and not _os.environ.get("K_NO_IDENT"):
                    # S^T += pair^T on the PE via identity accumulation
                    for half in range(2):
                        nc.tensor.matmul(
                            sp[:, 512 * half : 512 * (half + 1)],
                            ident_t,
                            pt_flat[:, 512 * half : 512 * (half + 1)],
                            start=False,
                            stop=True,
                            skip_group_check=True,
                        )
                else:
                    # S^T += pair^T on the DVE
                    nc.vector.tensor_add(sp, sp, pt_flat.bitcast(F32))
                e_t = exp_pool.tile([128, GK * QS], BF16, tag="E", name="E")
                nc.scalar.activation(out=e_t, in_=sp, func=mybir.ActivationFunctionType.Exp)
                return e_t

            def emit_av(i, e_t):
                h, g = steps[i]
                if g == 0:
                    ot_by_head[h] = otsum_pool.tile(
                        [CH + 1, QS], F32, tag="ot", name="ot"
                    )
                ot = ot_by_head[h]
                for j in range(GK):
                    c = GK * g + j
                    nc.tensor.matmul(
                        ot,
                        vhat[c][:, h, :],
                        e_t[:, QS * j : QS * (j + 1)],
                        start=(c == 0),
                        stop=(c == KC - 1),
                        skip_group_check=True,
                    )
                if g == NG - 1:
                    tail_queue.append(("recip", h))
                    tail_queue.append(("proj", h))

            def emit_tail(stage):
                kind, h = stage
                ot = ot_by_head[h]
                if kind == "recip":
                    # exact reciprocal: reciprocal_approx_* are custom DVE
                    # opcodes that crash this runtime (NRT_EXEC_UNIT_...).
                    r1f = head_pool.tile([CH + 1, QS], F32, tag="r1f", name="r")
                    nc.vector.reciprocal(
                        out=r1f[CH : CH + 1, :], in_=ot[CH : CH + 1, :]
                    )
                    r1 = head_pool.tile([CH + 1, QS], F32R, tag="r1", name="r1")
                    nc.vector.tensor_copy(
                        r1[CH : CH + 1, :], r32(r1f)[CH : CH + 1, :]
                    )
                    head_state[h] = r1
                else:
                    r1 = head_state[h]
                    rb = mmsum.tile([128, 1024], F32, tag="sp", name="ps")[
                        0:CH, 0:QS
                    ]
                    nc.tensor.matmul(
                        rb,
                        ones_t[CH : CH + 1, :],
                        r1[CH : CH + 1, :],
                        start=True,
                        stop=True,
                    )
                    gom = head_pool.tile([CH, QS], F32R, tag="gom", name="gom")
                    with nc.allow_low_precision(reason="f32r is fp32-width"):
                        nc.vector.tensor_mul(gom, ot[0:CH, :], gT[h])
                        nc.vector.tensor_mul(gom, gom, r32(rb))
                    for qc in range(QS // 128):
                        nc.tensor.matmul(
                            y_ps[:, 256 * qc : 256 * (qc + 1)],
                            gom[:, 128 * qc : 128 * (qc + 1)],
                            wo_h[h],
                            # only the very first matmul into this bank may set
                            # start (it clears has_written for the WHOLE bank)
                            start=(h == 0 and qc == 0),
                            stop=(h == H - 1),
                            skip_group_check=True,
                        )

            head_state = {}
            for i in range(len(steps)):
                e_t = emit_qk(i)
                if pending_av is not None:
                    emit_av(*pending_av)
                if tail_queue:
                    emit_tail(tail_queue.pop(0))
                pending_av = (i, e_t)
            emit_av(*pending_av)
            while tail_queue:
                emit_tail(tail_queue.pop(0))

            # ---- evacuate y ----
            for qc in range(QS // 128):
                ys = head_pool.tile([128, C], F32, tag="ys")
                nc.vector.tensor_copy(ys, y_ps[:, 256 * qc : 256 * (qc + 1)])
                nc.sync.dma_start(out=y[128 * qc : 128 * (qc + 1), :], in_=ys)

    nc.compile()
    return nc


_NC_CACHE = None


def get_nc():
    global _NC_CACHE
    if _NC_CACHE is None:
        _NC_CACHE = build_nc()
    return _NC_CACHE


def make_in_maps(q_x, kv_x, pair_bias, mask_bias, w_q, w_k, w_v, w_g, b_g, w_o):
    f = np.float32
    q_x = np.asarray(q_x, f)
    kv_x = np.asarray(kv_x, f)
    pair_bias = np.asarray(pair_bias, f)
    mask_bias = np.asarray(mask_bias, f)
    shared = {
        "kvxT": np.ascontiguousarray(kv_x[0].T),
        "wq": np.ascontiguousarray(np.asarray(w_q, f) / math.sqrt(CH)),
        "wk": np.ascontiguousarray(np.asarray(w_k, f)),
        "wv": np.ascontiguousarray(np.asarray(w_v, f)),
        "wg": np.ascontiguousarray(np.asarray(w_g, f)),
        "wo": np.ascontiguousarray(np.asarray(w_o, f)),
        "bgt": np.ascontiguousarray(np.asarray(b_g, f).reshape(H, CH).T),
        "maskT": np.ascontiguousarray(mask_bias.reshape(KC, 128).T),
        "ones": np.ones((1, CH), f),
        "ident": np.eye(128, dtype=f),
    }
    in_maps = []
    for i in range(NCORES):
        sl = slice(QS * i, QS * (i + 1))
        in_maps.append(
            dict(
                shared,
                pairT=np.ascontiguousarray(pair_bias[0, :, sl, :].transpose(0, 2, 1)),
                qxT=np.ascontiguousarray(q_x[0, sl, :].T),
            )
        )
    return in_maps


def kernel(
    q_x, kv_x, pair_bias, mask_bias, w_q, w_k, w_v, w_g, b_g, w_o, b_o, **run_kwargs
):
    nc = get_nc()
    in_maps = make_in_maps(
        q_x, kv_x, pair_bias, mask_bias, w_q, w_k, w_v, w_g, b_g, w_o
    )
    res = run_bass_kernel_spmd(nc, in_maps, core_ids=list(range(NCORES)), **run_kwargs)
    out = np.concatenate([res.results[i]["y"] for i in range(NCORES)], axis=0)
    out = out + np.asarray(b_o, np.float32)[None, :]
    kernel.last_result = res
    return out[None].astype(np.float32)
